# revision 35
# baseline (speedup 1.0000x reference)
"""Distributed Trainium2 kernel for nn_Attention_7722351198977 (v2).

Structure per core (8 cores, sequence-parallel with interleaved causal
sub-blocks for load balance):
  - core c owns 8 sub-blocks of 16 query rows: class k in {0..3}, sub-blocks
    {16k + c, 16k + c + 8}; class k's causal extent is 256*(k+1) columns.
  - everything streamed in fp16 (rel err budget allows it), fp32 PSUM accum.
  - k,v all-gathered across cores in fp16; relpos truncated to causal extent
    (10.5 MB/core instead of 16) and laid out host-side for contiguous DMA.
"""

import os as _os

import numpy as np

import concourse.bass as bass
import concourse.bacc as bacc
import concourse.mybir as mybir
import concourse.tile as tile
from concourse.bass_utils import run_bass_kernel_spmd
from concourse.masks import make_identity

F32 = mybir.dt.float32
F16 = mybir.dt.float16
B, T, D, H = 8, 1024, 1024, 64
NC = 8
TC = T // NC          # 128 query rows per core
NPAIR = TC // 2       # 64 row pairs per core
NEG = -30000.0        # mask value (fits in f16)

# classes: 4 causal-extent classes, ext = 256*(k+1)
EXTS = [256 * (k + 1) for k in range(4)]
# chunks: 16 chunks of 4 pairs; chunk i covers pairs 4i..4i+4, class i//4
CHUNK_EXT = [EXTS[i // 4] for i in range(16)]
# column offsets of each pair's block in the packed relp stream
PAIR_EXT = [EXTS[p // 16] for p in range(NPAIR)]
PAIR_OFF = np.concatenate([[0], np.cumsum(PAIR_EXT)]).astype(int)
RELP_COLS = int(PAIR_OFF[-1])  # 40960
# stage offsets (f16 elements): chunk i holds 128 rows x 8*Wk
CHUNK_W = [64 if i // 4 < 2 else 128 for i in range(16)]
ST_SIZE = [64 * 8 * w for w in CHUNK_W]
ST_OFF = np.concatenate([[0], np.cumsum(ST_SIZE)]).astype(int)
ST_TOT = int(ST_OFF[-1])

ATTN_DMA_T = _os.environ.get("ATTN_DMA_T", "1") == "1"
VT_DMA_T = _os.environ.get("VT_DMA_T", "1") == "1"
DEBUG = _os.environ.get("ATTN_DEBUG", "0") == "1"


def sub_blocks(core):
    """8 sub-block ids for a core, in stacking (group) order."""
    return [16 * (g // 2) + core + 8 * (g % 2) for g in range(8)]


def global_rows(core):
    rows = []
    for s in sub_blocks(core):
        rows.extend(range(16 * s, 16 * s + 16))
    return np.array(rows)


def build(num_cores: int = NC) -> bass.Bass:
    nc = bacc.Bacc(
        "TRN2", target_bir_lowering=False, debug=False, num_devices=num_cores
    )

    xT = nc.declare_dram_parameter("xT", [D, B * TC], F16, isOutput=False)
    wqkv = nc.declare_dram_parameter("wqkv", [D, 192], F16, isOutput=False)
    relp = nc.declare_dram_parameter("relp", [TC, RELP_COLS], F16, isOutput=False)
    mask = nc.declare_dram_parameter("mask", [TC, T], F16, isOutput=False)
    out_e = nc.declare_dram_parameter("out", [B * TC, H], F32, isOutput=True)

    stage = nc.dram_tensor("stage", [ST_TOT], F16)
    cc_in = nc.dram_tensor("cc_in", [TC, B * TC], F16)
    cc_out = nc.dram_tensor("cc_out", [NC * TC, B * TC], F16, addr_space="Shared")

    dbg = {}
    if DEBUG:
        for nm, shp in [
            ("d_qk", [128, B * TC]),
            ("d_vsb", [128, B * H]),
            ("d_ccout", [NC * TC, B * TC]),
            ("d_biastc", [128, B * T]),
            ("d_ae", [128, T]),
            ("d_aT", [128, T]),
        ]:
            dbg[nm] = nc.declare_dram_parameter(nm, shp, F16, isOutput=True)
        dbg["d_apre"] = nc.declare_dram_parameter(
            "d_apre", [128, T], F32, isOutput=True
        )

    Copy = mybir.ActivationFunctionType.Copy
    Exp = mybir.ActivationFunctionType.Exp

    with tile.TileContext(nc) as tc:
        with (
            tc.tile_pool(name="const", bufs=1) as constp,
            tc.tile_pool(name="xtp", bufs=1) as xtp,
            tc.tile_pool(name="rp0", bufs=2) as rp0,
            tc.tile_pool(name="rp1", bufs=2) as rp1,
            tc.tile_pool(name="rp2", bufs=2) as rp2,
            tc.tile_pool(name="rp3", bufs=2) as rp3,
            tc.tile_pool(name="bst", bufs=2) as bstp,
            tc.tile_pool(name="attn", bufs=2) as attnp,
            tc.tile_pool(name="small", bufs=8) as smallp,
            tc.tile_pool(
                name="psA", bufs=3 if (VT_DMA_T and ATTN_DMA_T) else 2,
                space="PSUM",
            ) as psA,
            tc.tile_pool(name="psO", bufs=2, space="PSUM") as psOp,
        ):
            rpools = [rp0, rp1, rp2, rp3]

            # ---- constants / weights ----
            wqkv_sb = constp.tile([128, 8, 192], F16)
            nc.sync.dma_start(
                out=wqkv_sb[:], in_=wqkv.rearrange("(c p) m -> p c m", p=128)
            )
            mask_sb = constp.tile([128, T], F16)
            nc.sync.dma_start(out=mask_sb[:], in_=mask[:])

            identf = constp.tile([128, 128], F32)
            make_identity(nc, identf[:])
            ident16 = constp.tile([128, 128], F16)
            nc.vector.tensor_copy(ident16[:], identf[:])

            qstage = constp.tile([128, NPAIR * 16], F16)
            bias_tc = constp.tile([128, B, T], F16)

            # ---- x load (4 tiles of 2 chunks for early matmul start) ----
            xts = []
            xv = xT.rearrange("(c p) r -> p c r", p=128)
            for i in range(4):
                xt = xtp.tile([128, 2, B * TC], F16, tag=f"xt{i}", name=f"xt{i}")
                nc.scalar.dma_start(out=xt[:], in_=xv[:, 2 * i : 2 * i + 2, :])
                xts.append(xt)

            # ---- q,k projections: psQK[128=(q|k), B*TC] ----
            psQK = psA.tile([128, B * TC], F32, tag="big", name="psQK")
            for h2 in range(2):
                sl = slice(512 * h2, 512 * h2 + 512)
                for c in range(8):
                    nc.tensor.matmul(
                        psQK[:, sl],
                        wqkv_sb[:, c, 0:128],
                        xts[c // 2][:, c % 2, sl],
                        start=(c == 0),
                        stop=(c == 7),
                    )
            qT16 = constp.tile([H, B * TC], F16)
            kT16 = constp.tile([H, B * TC], F16)
            nc.scalar.activation(qT16[:], psQK[0:H, :], Copy)
            nc.scalar.activation(kT16[:], psQK[H:128, :], Copy, scale=8.0)

            # ---- v projection: vT[64, B*TC] then transpose to [tl, b, h] ----
            psVT_t = psA.tile([128, B * TC], F32, tag="big", name="psVT")
            psVT = psVT_t[0:H]
            for h2 in range(2):
                sl = slice(512 * h2, 512 * h2 + 512)
                for c in range(8):
                    nc.tensor.matmul(
                        psVT[:, sl],
                        wqkv_sb[:, c, 128:192],
                        xts[c // 2][:, c % 2, sl],
                        start=(c == 0),
                        stop=(c == 7),
                    )
            vT16 = constp.tile([H, B * TC], F16)
            nc.scalar.activation(vT16[:], psVT, Copy)
            v_sb = constp.tile([128, B, H], F16)
            if VT_DMA_T:
                for b in range(B):
                    nc.sync.dma_start_transpose(
                        out=v_sb[:, b, :], in_=vT16[:, b * TC : (b + 1) * TC]
                    )
            else:
                for b in range(B):
                    psTr_t = psA.tile([128, B * TC], F32, tag="big", name=f"vtr{b}")
                    psTr = psTr_t.bitcast(F16)
                    nc.tensor.transpose(
                        psTr[:, 0:H], vT16[:, b * TC : (b + 1) * TC],
                        ident16[0:H, 0:H],
                    )
                    nc.scalar.activation(v_sb[:, b, :], psTr[:, 0:H], Copy)

            if DEBUG:
                nc.scalar.dma_start(out=dbg["d_qk"][0:H, :], in_=qT16[:])
                nc.scalar.dma_start(out=dbg["d_qk"][H:128, :], in_=kT16[:])
                nc.scalar.dma_start(
                    out=dbg["d_vsb"][:],
                    in_=v_sb[:].rearrange("p b h -> p (b h)"),
                )

            # ---- kick all-gather of (kT, v) in f16 ----
            nc.sync.dma_start(out=cc_in[0:H, :], in_=kT16[:])
            nc.sync.dma_start(
                out=cc_in[H:TC, :].rearrange("p (a bh) -> (p a) bh", a=2),
                in_=v_sb[:].rearrange("p b h -> p (b h)"),
            )
            nc.gpsimd.collective_compute(
                "AllGather",
                mybir.AluOpType.bypass,
                replica_groups=[list(range(num_cores))],
                ins=[cc_in[:]],
                outs=[cc_out[:]],
            )
            # memsets after CC emission so gpsimd reaches the collective
            # barrier as early as possible
            nc.gpsimd.memset(qstage[:], 0.0)
            nc.gpsimd.memset(bias_tc[:], 0.0)
            # post-CC loads on gpsimd: same engine as the collective, so they
            # are program-ordered after it (DRAM is not hazard-tracked).
            # Split per b so attention b can start as soon as its slice lands.
            kT_all = constp.tile([H, NC, B, TC], F16)
            v_nat = constp.tile([TC, NC, B, H], F16)
            cc_k = cc_out.rearrange("(j p) (b t) -> p j b t", j=NC, p=TC, b=B)
            cc_v = cc_out.rearrange(
                "(j w p) (a b h) -> (p a) j w b h", j=NC, w=2, p=H, a=2, b=B
            )
            nc.gpsimd.dma_start(out=kT_all[:], in_=cc_k[0:H])
            nc.gpsimd.dma_start(out=v_nat[:], in_=cc_v[:, :, 1])

            if DEBUG:
                nc.scalar.dma_start(out=dbg["d_ccout"][:], in_=cc_out[:])

            # ---- qstage: col(p,s,b) = q[b, row 2p+s], partitions s*64..+64 ----
            qsrc = qT16.rearrange("h (b p s) -> h p s b", b=B, p=NPAIR, s=2)
            qdst_lo = qstage[0:H, :].rearrange("h (p s b) -> h p s b", p=NPAIR, s=2, b=B)
            qdst_hi = qstage[H:128, :].rearrange("h (p s b) -> h p s b", p=NPAIR, s=2, b=B)
            nc.vector.tensor_copy(qdst_lo[:, :, 0, :], qsrc[:, :, 0, :])
            nc.vector.tensor_copy(qdst_hi[:, :, 1, :], qsrc[:, :, 1, :])

            # ---- bias phase: 16 chunks of 4 pairs ----
            for i in range(16):
                ext = CHUNK_EXT[i]
                rchunk = rpools[i // 4].tile([128, 4, ext], F16, tag=f"rc{i // 4}")
                c0 = int(PAIR_OFF[4 * i])
                nc.sync.dma_start(
                    out=rchunk[:],
                    in_=relp[:, c0 : c0 + 4 * ext].rearrange(
                        "p (f e) -> p f e", f=4
                    ),
                )
                psBias = psA.tile([128, B * TC], F32, tag="big", name=f"psB{i}")
                for p4 in range(4):
                    p = 4 * i + p4
                    for e0 in range(0, ext, 512):
                        e1 = min(ext, e0 + 512)
                        nc.tensor.matmul(
                            psBias[32 * p4 : 32 * p4 + 16, e0:e1],
                            qstage[:, 16 * p : 16 * p + 16],
                            rchunk[:, p4, e0:e1],
                            tile_position=(0, 32 * p4),
                            start=True,
                            stop=True,
                        )
                # quad copy with j-slot scatter on DVE (slot width W per
                # class so attention does wide uniform adds)
                kcl = i // 4
                Wk = 64 if kcl < 2 else 128
                tk = 32 * (kcl + 1)
                bias_sb = bstp.tile([128, T], F16, tag="bsb")
                if i < 2:
                    # first use of each buf: zero (uninit SBUF may hold NaN
                    # bit patterns; later stale data is finite and masked)
                    nc.gpsimd.memset(bias_sb[:], 0.0)
                nc.vector.tensor_copy(
                    bias_sb[:, 0 : 8 * Wk].rearrange("u (j t) -> u j t", j=NC)[
                        :, :, 0:tk
                    ],
                    psBias[:, 0:ext].rearrange("u (j t) -> u j t", j=NC),
                )
                # partition redistribution via compact DRAM stage (only the
                # 64 useful band rows). All on one queue: DRAM write->read
                # ordering relies on queue FIFO.
                r0 = 8 * i
                o = int(ST_OFF[i])
                st_v = stage[o : o + 64 * 8 * Wk].rearrange(
                    "(p4 s b v) -> p4 s b v", p4=4, s=2, b=B
                )
                for p4 in range(4):
                    nc.scalar.dma_start(
                        out=st_v[p4].rearrange("s b v -> (s b) v"),
                        in_=bias_sb[32 * p4 : 32 * p4 + 16, 0 : 8 * Wk],
                    )
                for p4 in range(4):
                    nc.scalar.dma_start(
                        out=bias_tc[r0 + 2 * p4 : r0 + 2 * p4 + 2, :, 0 : 8 * Wk],
                        in_=st_v[p4],
                    )

            if DEBUG:
                nc.scalar.dma_start(
                    out=dbg["d_biastc"][:],
                    in_=bias_tc[:].rearrange("p b v -> p (b v)"),
                )

            # ---- attention phase: per batch b ----
            for b in range(B):
                psS = psA.tile([128, T], F32, tag="big", name=f"psS{b}")
                # preload causal mask into PSUM; scores accumulate on top
                nc.scalar.activation(psS[:], mask_sb[:], Copy)
                for h2 in range(2):
                    nc.tensor.matmul(
                        psS[:, 512 * h2 : 512 * h2 + 512],
                        qT16[:, b * TC : (b + 1) * TC],
                        kT_all[:, 4 * h2 : 4 * h2 + 4, b, :],
                        start=False,
                        stop=True,
                        skip_group_check=True,
                    )
                # bias add in PSUM: lower rows slot width 64, upper full 128
                psS_v = psS.rearrange("p (j t) -> p j t", j=NC)
                nc.vector.tensor_tensor(
                    out=psS_v[0:64, :, 0:64],
                    in0=psS_v[0:64, :, 0:64],
                    in1=bias_tc[0:64, b, 0:512].rearrange(
                        "p (j t) -> p j t", j=NC
                    ),
                    op=mybir.AluOpType.add,
                )
                nc.vector.tensor_tensor(
                    out=psS[64:128, :],
                    in0=psS[64:128, :],
                    in1=bias_tc[64:128, b, :],
                    op=mybir.AluOpType.add,
                )
                negmax = smallp.tile([128, 1], F32, tag="nmax")
                nc.vector.reduce_max(
                    negmax[:], psS[:], axis=mybir.AxisListType.X, negate=True
                )
                attn_e = attnp.tile([128, T], F16, tag="aexp")
                denom = smallp.tile([128, 1], F32, tag="den")
                nc.scalar.activation(
                    attn_e[:], psS[:], Exp,
                    bias=negmax[:], scale=1.0, accum_out=denom[:],
                )
                attnT = attnp.tile([128, T], F16, tag="aT")
                if ATTN_DMA_T:
                    nc.sync.dma_start_transpose(
                        out=attnT[:].rearrange("p (j f) -> p j f", j=NC),
                        in_=attn_e[:],
                    )
                else:
                    for g2 in range(2):
                        psT_t = psA.tile([128, B * TC], F32, tag="big", name=f"pt{b}{g2}")
                        psT = psT_t.bitcast(F16)[:, 0:512]
                        for s4 in range(4):
                            j = 4 * g2 + s4
                            nc.tensor.transpose(
                                psT[:, 128 * s4 : 128 * s4 + 128],
                                attn_e[:, 128 * j : 128 * j + 128],
                                ident16[:],
                            )
                        nc.scalar.activation(
                            attnT[:, 512 * g2 : 512 * g2 + 512], psT[:], Copy
                        )
                if DEBUG and b == 0:
                    nc.scalar.dma_start(out=dbg["d_ae"][:], in_=attn_e[:])
                    nc.scalar.dma_start(out=dbg["d_aT"][:], in_=attnT[:])
                psO = psOp.tile([128, H], F32, tag="o", name=f"psO{b}")
                for j in range(8):
                    nc.tensor.matmul(
                        psO[:],
                        attnT[:, 128 * j : 128 * j + 128],
                        v_nat[:, j, b, :],
                        start=(j == 0),
                        stop=(j == 7),
                    )
                rden = smallp.tile([128, 1], F32, tag="rden")
                nc.vector.reciprocal(rden[:], denom[:])
                out_sb = smallp.tile([128, H], F32, tag="osb")
                nc.scalar.activation(out_sb[:], psO[:], Copy, scale=rden[:])
                nc.sync.dma_start(
                    out=out_e[b * TC : (b + 1) * TC, :], in_=out_sb[:]
                )
    nc.compile()
    return nc


_CACHE: dict = {}


def _get_nc():
    if "nc" not in _CACHE:
        _CACHE["nc"] = build(NC)
    return _CACHE["nc"]


def _prep_inputs(x, Wq, Wk, Wv, relpos):
    x16 = np.asarray(x, dtype=np.float16)
    rel16 = np.asarray(relpos, dtype=np.float16)
    wqkv = np.ascontiguousarray(
        np.concatenate([Wq, Wk, Wv], axis=1), dtype=np.float16
    )
    # column order of the gathered k/v axis: col j*128+tl -> global_rows(j)[tl]
    all_rows = [global_rows(j) for j in range(NC)]
    vperm = np.concatenate(all_rows)
    # per class k, the packed bias columns: concat_j global_rows(j)[:32*(k+1)]
    vpack = [
        np.concatenate([all_rows[j][: 32 * (k + 1)] for j in range(NC)])
        for k in range(4)
    ]
    in_maps = []
    for c in range(NC):
        rows = all_rows[c]
        xs = x16[:, rows, :]  # [B, 128, D]
        xT = np.ascontiguousarray(xs.transpose(2, 0, 1).reshape(D, B * TC))
        # relp stream: per pair p, [128, ext] = [R_tA^T ; R_tB^T] with columns
        # in the packed per-class order (prefix of each core's shard)
        rp = np.zeros((TC, RELP_COLS), dtype=np.float16)
        for p in range(NPAIR):
            tA = int(rows[2 * p])
            tB = int(rows[2 * p + 1])
            vr = vpack[p // 16]
            o = int(PAIR_OFF[p])
            e = PAIR_EXT[p]
            rp[0:H, o : o + e] = rel16[tA, vr, :].T
            rp[H:128, o : o + e] = rel16[tB, vr, :].T
        # mask in gathered column order
        msk = np.where(vperm[None, :] <= rows[:, None], 0.0, NEG).astype(
            np.float16
        )
        in_maps.append({"xT": xT, "wqkv": wqkv, "relp": rp, "mask": msk})
    return in_maps


def run_sharded(in_maps, trace=False, **kw):
    nc = _get_nc()
    return run_bass_kernel_spmd(
        nc, in_maps, core_ids=list(range(NC)), trace=trace, **kw
    )


def kernel(x, Wq, Wk, Wv, relpos):
    in_maps = _prep_inputs(x, Wq, Wk, Wv, relpos)
    res = run_sharded(in_maps, trace=False)
    out = np.empty((B, T, H), dtype=np.float32)
    for c in range(NC):
        rows = global_rows(c)
        out[:, rows, :] = res.results[c]["out"].reshape(B, TC, H)
    return out


# revision 36
# speedup vs baseline: 1.1610x; 1.1610x over previous
"""Distributed Trainium2 kernel for nn_Attention_7722351198977 (v2).

Structure per core (8 cores, sequence-parallel with interleaved causal
sub-blocks for load balance):
  - core c owns 8 sub-blocks of 16 query rows: class k in {0..3}, sub-blocks
    {16k + c, 16k + c + 8}; class k's causal extent is 256*(k+1) columns.
  - everything streamed in fp16 (rel err budget allows it), fp32 PSUM accum.
  - k,v all-gathered across cores in fp16; relpos truncated to causal extent
    (10.5 MB/core instead of 16) and laid out host-side for contiguous DMA.
"""

import os as _os

import numpy as np

import concourse.bass as bass
import concourse.bacc as bacc
import concourse.mybir as mybir
import concourse.tile as tile
from concourse.bass_utils import run_bass_kernel_spmd
from concourse.masks import make_identity

F32 = mybir.dt.float32
F16 = mybir.dt.float16
B, T, D, H = 8, 1024, 1024, 64
NC = 8
TC = T // NC          # 128 query rows per core
NPAIR = TC // 2       # 64 row pairs per core
NEG = -30000.0        # mask value (fits in f16)

# classes: 4 causal-extent classes, ext = 256*(k+1)
EXTS = [256 * (k + 1) for k in range(4)]
# chunks: 16 chunks of 4 pairs; chunk i covers pairs 4i..4i+4, class i//4
CHUNK_EXT = [EXTS[i // 4] for i in range(16)]
# column offsets of each pair's block in the packed relp stream
PAIR_EXT = [EXTS[p // 16] for p in range(NPAIR)]
PAIR_OFF = np.concatenate([[0], np.cumsum(PAIR_EXT)]).astype(int)
RELP_COLS = int(PAIR_OFF[-1])  # 40960
# stage offsets (f16 elements): chunk i holds 128 rows x 8*Wk
CHUNK_W = [64 if i // 4 < 2 else 128 for i in range(16)]
ST_SIZE = [128 * 8 * w for w in CHUNK_W]
ST_OFF = np.concatenate([[0], np.cumsum(ST_SIZE)]).astype(int)
ST_TOT = int(ST_OFF[-1])

ATTN_DMA_T = _os.environ.get("ATTN_DMA_T", "1") == "1"
VT_DMA_T = _os.environ.get("VT_DMA_T", "1") == "1"
DEBUG = _os.environ.get("ATTN_DEBUG", "0") == "1"


def sub_blocks(core):
    """8 sub-block ids for a core, in stacking (group) order."""
    return [16 * (g // 2) + core + 8 * (g % 2) for g in range(8)]


def global_rows(core):
    rows = []
    for s in sub_blocks(core):
        rows.extend(range(16 * s, 16 * s + 16))
    return np.array(rows)


def build(num_cores: int = NC) -> bass.Bass:
    nc = bacc.Bacc(
        "TRN2", target_bir_lowering=False, debug=False, num_devices=num_cores
    )

    xT = nc.declare_dram_parameter("xT", [D, B * TC], F16, isOutput=False)
    wqkv = nc.declare_dram_parameter("wqkv", [D, 192], F16, isOutput=False)
    relp = nc.declare_dram_parameter("relp", [TC, RELP_COLS], F16, isOutput=False)
    mask = nc.declare_dram_parameter("mask", [TC, T], F16, isOutput=False)
    out_e = nc.declare_dram_parameter("out", [B * TC, H], F32, isOutput=True)

    stage = nc.dram_tensor("stage", [ST_TOT], F16)
    cc_in = nc.dram_tensor("cc_in", [TC, B * TC], F16)
    cc_out = nc.dram_tensor("cc_out", [NC * TC, B * TC], F16, addr_space="Shared")

    dbg = {}
    if DEBUG:
        for nm, shp in [
            ("d_qk", [128, B * TC]),
            ("d_vsb", [128, B * H]),
            ("d_ccout", [NC * TC, B * TC]),
            ("d_biastc", [128, B * T]),
            ("d_ae", [128, T]),
            ("d_aT", [128, T]),
        ]:
            dbg[nm] = nc.declare_dram_parameter(nm, shp, F16, isOutput=True)
        dbg["d_apre"] = nc.declare_dram_parameter(
            "d_apre", [128, T], F32, isOutput=True
        )

    Copy = mybir.ActivationFunctionType.Copy
    Exp = mybir.ActivationFunctionType.Exp

    with tile.TileContext(nc) as tc:
        with (
            tc.tile_pool(name="const", bufs=1) as constp,
            tc.tile_pool(name="xtp", bufs=1) as xtp,
            tc.tile_pool(name="rp0", bufs=2) as rp0,
            tc.tile_pool(name="rp1", bufs=2) as rp1,
            tc.tile_pool(name="rp2", bufs=2) as rp2,
            tc.tile_pool(name="rp3", bufs=2) as rp3,
            tc.tile_pool(name="bst", bufs=2) as bstp,
            tc.tile_pool(name="attn", bufs=2) as attnp,
            tc.tile_pool(name="small", bufs=8) as smallp,
            tc.tile_pool(
                name="psA", bufs=3 if (VT_DMA_T and ATTN_DMA_T) else 2,
                space="PSUM",
            ) as psA,
            tc.tile_pool(name="psO", bufs=2, space="PSUM") as psOp,
        ):
            rpools = [rp0, rp1, rp2, rp3]

            # ---- constants / weights ----
            wqkv_sb = constp.tile([128, 8, 192], F16)
            nc.sync.dma_start(
                out=wqkv_sb[:], in_=wqkv.rearrange("(c p) m -> p c m", p=128)
            )
            mask_sb = constp.tile([128, T], F16)
            nc.sync.dma_start(out=mask_sb[:], in_=mask[:])

            identf = constp.tile([128, 128], F32)
            make_identity(nc, identf[:])
            ident16 = constp.tile([128, 128], F16)
            nc.vector.tensor_copy(ident16[:], identf[:])

            qstage = constp.tile([128, NPAIR * 16], F16)
            bias_tc = constp.tile([128, B, T], F16)

            # ---- x load (4 tiles of 2 chunks for early matmul start) ----
            xts = []
            xv = xT.rearrange("(c p) r -> p c r", p=128)
            for i in range(4):
                xt = xtp.tile([128, 2, B * TC], F16, tag=f"xt{i}", name=f"xt{i}")
                nc.scalar.dma_start(out=xt[:], in_=xv[:, 2 * i : 2 * i + 2, :])
                xts.append(xt)

            # ---- q,k projections: psQK[128=(q|k), B*TC] ----
            psQK = psA.tile([128, B * TC], F32, tag="big", name="psQK")
            for h2 in range(2):
                sl = slice(512 * h2, 512 * h2 + 512)
                for c in range(8):
                    nc.tensor.matmul(
                        psQK[:, sl],
                        wqkv_sb[:, c, 0:128],
                        xts[c // 2][:, c % 2, sl],
                        start=(c == 0),
                        stop=(c == 7),
                    )
            qT16 = constp.tile([H, B * TC], F16)
            kT16 = constp.tile([H, B * TC], F16)
            nc.scalar.activation(qT16[:], psQK[0:H, :], Copy)
            nc.scalar.activation(kT16[:], psQK[H:128, :], Copy, scale=8.0)

            # ---- v projection: vT[64, B*TC] then transpose to [tl, b, h] ----
            psVT_t = psA.tile([128, B * TC], F32, tag="big", name="psVT")
            psVT = psVT_t[0:H]
            for h2 in range(2):
                sl = slice(512 * h2, 512 * h2 + 512)
                for c in range(8):
                    nc.tensor.matmul(
                        psVT[:, sl],
                        wqkv_sb[:, c, 128:192],
                        xts[c // 2][:, c % 2, sl],
                        start=(c == 0),
                        stop=(c == 7),
                    )
            vT16 = constp.tile([H, B * TC], F16)
            nc.scalar.activation(vT16[:], psVT, Copy)
            v_sb = constp.tile([128, B, H], F16)
            if VT_DMA_T:
                for b in range(B):
                    nc.sync.dma_start_transpose(
                        out=v_sb[:, b, :], in_=vT16[:, b * TC : (b + 1) * TC]
                    )
            else:
                for b in range(B):
                    psTr_t = psA.tile([128, B * TC], F32, tag="big", name=f"vtr{b}")
                    psTr = psTr_t.bitcast(F16)
                    nc.tensor.transpose(
                        psTr[:, 0:H], vT16[:, b * TC : (b + 1) * TC],
                        ident16[0:H, 0:H],
                    )
                    nc.scalar.activation(v_sb[:, b, :], psTr[:, 0:H], Copy)

            if DEBUG:
                nc.scalar.dma_start(out=dbg["d_qk"][0:H, :], in_=qT16[:])
                nc.scalar.dma_start(out=dbg["d_qk"][H:128, :], in_=kT16[:])
                nc.scalar.dma_start(
                    out=dbg["d_vsb"][:],
                    in_=v_sb[:].rearrange("p b h -> p (b h)"),
                )

            # ---- kick all-gather of (kT, v) in f16 ----
            nc.sync.dma_start(out=cc_in[0:H, :], in_=kT16[:])
            nc.sync.dma_start(
                out=cc_in[H:TC, :].rearrange("p (a bh) -> (p a) bh", a=2),
                in_=v_sb[:].rearrange("p b h -> p (b h)"),
            )
            nc.gpsimd.collective_compute(
                "AllGather",
                mybir.AluOpType.bypass,
                replica_groups=[list(range(num_cores))],
                ins=[cc_in[:]],
                outs=[cc_out[:]],
            )
            # memsets after CC emission so gpsimd reaches the collective
            # barrier as early as possible
            nc.gpsimd.memset(qstage[:], 0.0)
            nc.gpsimd.memset(bias_tc[:], 0.0)
            # post-CC loads on gpsimd: same engine as the collective, so they
            # are program-ordered after it (DRAM is not hazard-tracked).
            # Split per b so attention b can start as soon as its slice lands.
            kT_all = constp.tile([H, NC, B, TC], F16)
            v_nat = constp.tile([TC, NC, B, H], F16)
            cc_k = cc_out.rearrange("(j p) (b t) -> p j b t", j=NC, p=TC, b=B)
            cc_v = cc_out.rearrange(
                "(j w p) (a b h) -> (p a) j w b h", j=NC, w=2, p=H, a=2, b=B
            )
            nc.gpsimd.dma_start(out=kT_all[:], in_=cc_k[0:H])
            nc.gpsimd.dma_start(out=v_nat[:], in_=cc_v[:, :, 1])

            if DEBUG:
                nc.scalar.dma_start(out=dbg["d_ccout"][:], in_=cc_out[:])

            # ---- qstage: col(p,s,b) = q[b, row 2p+s], partitions s*64..+64 ----
            qsrc = qT16.rearrange("h (b p s) -> h p s b", b=B, p=NPAIR, s=2)
            qdst_lo = qstage[0:H, :].rearrange("h (p s b) -> h p s b", p=NPAIR, s=2, b=B)
            qdst_hi = qstage[H:128, :].rearrange("h (p s b) -> h p s b", p=NPAIR, s=2, b=B)
            nc.vector.tensor_copy(qdst_lo[:, :, 0, :], qsrc[:, :, 0, :])
            nc.vector.tensor_copy(qdst_hi[:, :, 1, :], qsrc[:, :, 1, :])

            # ---- bias phase: 16 chunks of 4 pairs ----
            for i in range(16):
                ext = CHUNK_EXT[i]
                rchunk = rpools[i // 4].tile([128, 4, ext], F16, tag=f"rc{i // 4}")
                c0 = int(PAIR_OFF[4 * i])
                nc.sync.dma_start(
                    out=rchunk[:],
                    in_=relp[:, c0 : c0 + 4 * ext].rearrange(
                        "p (f e) -> p f e", f=4
                    ),
                )
                psBias = psA.tile([128, B * TC], F32, tag="big", name=f"psB{i}")
                for p4 in range(4):
                    p = 4 * i + p4
                    for e0 in range(0, ext, 512):
                        e1 = min(ext, e0 + 512)
                        nc.tensor.matmul(
                            psBias[32 * p4 : 32 * p4 + 16, e0:e1],
                            qstage[:, 16 * p : 16 * p + 16],
                            rchunk[:, p4, e0:e1],
                            tile_position=(0, 32 * p4),
                            start=True,
                            stop=True,
                        )
                # quad copy with j-slot scatter on DVE (slot width W per
                # class so attention does wide uniform adds)
                kcl = i // 4
                Wk = 64 if kcl < 2 else 128
                tk = 32 * (kcl + 1)
                bias_sb = bstp.tile([128, T], F16, tag="bsb")
                if i < 2:
                    # first use of each buf: zero (uninit SBUF may hold NaN
                    # bit patterns; later stale data is finite and masked)
                    nc.gpsimd.memset(bias_sb[:], 0.0)
                nc.vector.tensor_copy(
                    bias_sb[:, 0 : 8 * Wk].rearrange("u (j t) -> u j t", j=NC)[
                        :, :, 0:tk
                    ],
                    psBias[:, 0:ext].rearrange("u (j t) -> u j t", j=NC),
                )
                # partition redistribution via compact DRAM stage (only the
                # 64 useful band rows). All on one queue: DRAM write->read
                # ordering relies on queue FIFO.
                r0 = 8 * i
                o = int(ST_OFF[i])
                nc.scalar.dma_start(
                    out=stage[o : o + 128 * 8 * Wk].rearrange(
                        "(r v) -> r v", r=128
                    ),
                    in_=bias_sb[:, 0 : 8 * Wk],
                )
                st_v = stage[o : o + 128 * 8 * Wk].rearrange(
                    "(u s b v) -> u s b v", u=8, s=2, b=B
                )
                for p4 in range(4):
                    nc.scalar.dma_start(
                        out=bias_tc[r0 + 2 * p4 : r0 + 2 * p4 + 2, :, 0 : 8 * Wk],
                        in_=st_v[2 * p4],
                    )

            if DEBUG:
                nc.scalar.dma_start(
                    out=dbg["d_biastc"][:],
                    in_=bias_tc[:].rearrange("p b v -> p (b v)"),
                )

            # ---- attention phase: per batch b ----
            for b in range(B):
                psS = psA.tile([128, T], F32, tag="big", name=f"psS{b}")
                # preload causal mask into PSUM; scores accumulate on top
                nc.scalar.activation(psS[:], mask_sb[:], Copy)
                for h2 in range(2):
                    nc.tensor.matmul(
                        psS[:, 512 * h2 : 512 * h2 + 512],
                        qT16[:, b * TC : (b + 1) * TC],
                        kT_all[:, 4 * h2 : 4 * h2 + 4, b, :],
                        start=False,
                        stop=True,
                        skip_group_check=True,
                    )
                # bias add in PSUM: lower rows slot width 64, upper full 128
                psS_v = psS.rearrange("p (j t) -> p j t", j=NC)
                nc.vector.tensor_tensor(
                    out=psS_v[0:64, :, 0:64],
                    in0=psS_v[0:64, :, 0:64],
                    in1=bias_tc[0:64, b, 0:512].rearrange(
                        "p (j t) -> p j t", j=NC
                    ),
                    op=mybir.AluOpType.add,
                )
                nc.vector.tensor_tensor(
                    out=psS[64:128, :],
                    in0=psS[64:128, :],
                    in1=bias_tc[64:128, b, :],
                    op=mybir.AluOpType.add,
                )
                negmax = smallp.tile([128, 1], F32, tag="nmax")
                nc.vector.reduce_max(
                    negmax[:], psS[:], axis=mybir.AxisListType.X, negate=True
                )
                attn_e = attnp.tile([128, T], F16, tag="aexp")
                denom = smallp.tile([128, 1], F32, tag="den")
                nc.scalar.activation(
                    attn_e[:], psS[:], Exp,
                    bias=negmax[:], scale=1.0, accum_out=denom[:],
                )
                attnT = attnp.tile([128, T], F16, tag="aT")
                if ATTN_DMA_T:
                    nc.sync.dma_start_transpose(
                        out=attnT[:].rearrange("p (j f) -> p j f", j=NC),
                        in_=attn_e[:],
                    )
                else:
                    for g2 in range(2):
                        psT_t = psA.tile([128, B * TC], F32, tag="big", name=f"pt{b}{g2}")
                        psT = psT_t.bitcast(F16)[:, 0:512]
                        for s4 in range(4):
                            j = 4 * g2 + s4
                            nc.tensor.transpose(
                                psT[:, 128 * s4 : 128 * s4 + 128],
                                attn_e[:, 128 * j : 128 * j + 128],
                                ident16[:],
                            )
                        nc.scalar.activation(
                            attnT[:, 512 * g2 : 512 * g2 + 512], psT[:], Copy
                        )
                if DEBUG and b == 0:
                    nc.scalar.dma_start(out=dbg["d_ae"][:], in_=attn_e[:])
                    nc.scalar.dma_start(out=dbg["d_aT"][:], in_=attnT[:])
                psO = psOp.tile([128, H], F32, tag="o", name=f"psO{b}")
                for j in range(8):
                    nc.tensor.matmul(
                        psO[:],
                        attnT[:, 128 * j : 128 * j + 128],
                        v_nat[:, j, b, :],
                        start=(j == 0),
                        stop=(j == 7),
                    )
                rden = smallp.tile([128, 1], F32, tag="rden")
                nc.vector.reciprocal(rden[:], denom[:])
                out_sb = smallp.tile([128, H], F32, tag="osb")
                nc.scalar.activation(out_sb[:], psO[:], Copy, scale=rden[:])
                nc.sync.dma_start(
                    out=out_e[b * TC : (b + 1) * TC, :], in_=out_sb[:]
                )
    nc.compile()
    return nc


_CACHE: dict = {}


def _get_nc():
    if "nc" not in _CACHE:
        _CACHE["nc"] = build(NC)
    return _CACHE["nc"]


def _prep_inputs(x, Wq, Wk, Wv, relpos):
    x16 = np.asarray(x, dtype=np.float16)
    rel16 = np.asarray(relpos, dtype=np.float16)
    wqkv = np.ascontiguousarray(
        np.concatenate([Wq, Wk, Wv], axis=1), dtype=np.float16
    )
    # column order of the gathered k/v axis: col j*128+tl -> global_rows(j)[tl]
    all_rows = [global_rows(j) for j in range(NC)]
    vperm = np.concatenate(all_rows)
    # per class k, the packed bias columns: concat_j global_rows(j)[:32*(k+1)]
    vpack = [
        np.concatenate([all_rows[j][: 32 * (k + 1)] for j in range(NC)])
        for k in range(4)
    ]
    in_maps = []
    for c in range(NC):
        rows = all_rows[c]
        xs = x16[:, rows, :]  # [B, 128, D]
        xT = np.ascontiguousarray(xs.transpose(2, 0, 1).reshape(D, B * TC))
        # relp stream: per pair p, [128, ext] = [R_tA^T ; R_tB^T] with columns
        # in the packed per-class order (prefix of each core's shard)
        rp = np.zeros((TC, RELP_COLS), dtype=np.float16)
        for p in range(NPAIR):
            tA = int(rows[2 * p])
            tB = int(rows[2 * p + 1])
            vr = vpack[p // 16]
            o = int(PAIR_OFF[p])
            e = PAIR_EXT[p]
            rp[0:H, o : o + e] = rel16[tA, vr, :].T
            rp[H:128, o : o + e] = rel16[tB, vr, :].T
        # mask in gathered column order
        msk = np.where(vperm[None, :] <= rows[:, None], 0.0, NEG).astype(
            np.float16
        )
        in_maps.append({"xT": xT, "wqkv": wqkv, "relp": rp, "mask": msk})
    return in_maps


def run_sharded(in_maps, trace=False, **kw):
    nc = _get_nc()
    return run_bass_kernel_spmd(
        nc, in_maps, core_ids=list(range(NC)), trace=trace, **kw
    )


def kernel(x, Wq, Wk, Wv, relpos):
    in_maps = _prep_inputs(x, Wq, Wk, Wv, relpos)
    res = run_sharded(in_maps, trace=False)
    out = np.empty((B, T, H), dtype=np.float32)
    for c in range(NC):
        rows = global_rows(c)
        out[:, rows, :] = res.results[c]["out"].reshape(B, TC, H)
    return out


# revision 37
# speedup vs baseline: 1.1645x; 1.0030x over previous
"""Distributed Trainium2 kernel for nn_Attention_7722351198977 (v2).

Structure per core (8 cores, sequence-parallel with interleaved causal
sub-blocks for load balance):
  - core c owns 8 sub-blocks of 16 query rows: class k in {0..3}, sub-blocks
    {16k + c, 16k + c + 8}; class k's causal extent is 256*(k+1) columns.
  - everything streamed in fp16 (rel err budget allows it), fp32 PSUM accum.
  - k,v all-gathered across cores in fp16; relpos truncated to causal extent
    (10.5 MB/core instead of 16) and laid out host-side for contiguous DMA.
"""

import os as _os

import numpy as np

import concourse.bass as bass
import concourse.bacc as bacc
import concourse.mybir as mybir
import concourse.tile as tile
from concourse.bass_utils import run_bass_kernel_spmd
from concourse.masks import make_identity

F32 = mybir.dt.float32
F16 = mybir.dt.float16
B, T, D, H = 8, 1024, 1024, 64
NC = 8
TC = T // NC          # 128 query rows per core
NPAIR = TC // 2       # 64 row pairs per core
NEG = -30000.0        # mask value (fits in f16)

# classes: 4 causal-extent classes, ext = 256*(k+1)
EXTS = [256 * (k + 1) for k in range(4)]
# chunks: 16 chunks of 4 pairs; chunk i covers pairs 4i..4i+4, class i//4
CHUNK_EXT = [EXTS[i // 4] for i in range(16)]
# column offsets of each pair's block in the packed relp stream
PAIR_EXT = [EXTS[p // 16] for p in range(NPAIR)]
PAIR_OFF = np.concatenate([[0], np.cumsum(PAIR_EXT)]).astype(int)
RELP_COLS = int(PAIR_OFF[-1])  # 40960
# stage offsets (f16 elements): chunk i holds 128 rows x 8*Wk
CHUNK_W = [64 if i // 4 < 2 else 128 for i in range(16)]
ST_SIZE = [128 * 8 * w for w in CHUNK_W]
ST_OFF = np.concatenate([[0], np.cumsum(ST_SIZE)]).astype(int)
ST_TOT = int(ST_OFF[-1])

ATTN_DMA_T = _os.environ.get("ATTN_DMA_T", "0") == "1"
VT_DMA_T = _os.environ.get("VT_DMA_T", "1") == "1"
DEBUG = _os.environ.get("ATTN_DEBUG", "0") == "1"


def sub_blocks(core):
    """8 sub-block ids for a core, in stacking (group) order."""
    return [16 * (g // 2) + core + 8 * (g % 2) for g in range(8)]


def global_rows(core):
    rows = []
    for s in sub_blocks(core):
        rows.extend(range(16 * s, 16 * s + 16))
    return np.array(rows)


def build(num_cores: int = NC) -> bass.Bass:
    nc = bacc.Bacc(
        "TRN2", target_bir_lowering=False, debug=False, num_devices=num_cores
    )

    xT = nc.declare_dram_parameter("xT", [D, B * TC], F16, isOutput=False)
    wqkv = nc.declare_dram_parameter("wqkv", [D, 192], F16, isOutput=False)
    relp = nc.declare_dram_parameter("relp", [TC, RELP_COLS], F16, isOutput=False)
    mask = nc.declare_dram_parameter("mask", [TC, T], F16, isOutput=False)
    out_e = nc.declare_dram_parameter("out", [B * TC, H], F32, isOutput=True)

    stage = nc.dram_tensor("stage", [ST_TOT], F16)
    cc_in = nc.dram_tensor("cc_in", [TC, B * TC], F16)
    cc_out = nc.dram_tensor("cc_out", [NC * TC, B * TC], F16, addr_space="Shared")

    dbg = {}
    if DEBUG:
        for nm, shp in [
            ("d_qk", [128, B * TC]),
            ("d_vsb", [128, B * H]),
            ("d_ccout", [NC * TC, B * TC]),
            ("d_biastc", [128, B * T]),
            ("d_ae", [128, T]),
            ("d_aT", [128, T]),
        ]:
            dbg[nm] = nc.declare_dram_parameter(nm, shp, F16, isOutput=True)
        dbg["d_apre"] = nc.declare_dram_parameter(
            "d_apre", [128, T], F32, isOutput=True
        )

    Copy = mybir.ActivationFunctionType.Copy
    Exp = mybir.ActivationFunctionType.Exp

    with tile.TileContext(nc) as tc:
        with (
            tc.tile_pool(name="const", bufs=1) as constp,
            tc.tile_pool(name="xtp", bufs=1) as xtp,
            tc.tile_pool(name="rp0", bufs=2) as rp0,
            tc.tile_pool(name="rp1", bufs=2) as rp1,
            tc.tile_pool(name="rp2", bufs=2) as rp2,
            tc.tile_pool(name="rp3", bufs=2) as rp3,
            tc.tile_pool(name="bst", bufs=2) as bstp,
            tc.tile_pool(name="attn", bufs=2) as attnp,
            tc.tile_pool(name="small", bufs=8) as smallp,
            tc.tile_pool(name="psA", bufs=2, space="PSUM") as psA,
            tc.tile_pool(name="psT", bufs=2, space="PSUM") as psTp,
            tc.tile_pool(name="psO", bufs=2, space="PSUM") as psOp,
        ):
            rpools = [rp0, rp1, rp2, rp3]

            # ---- constants / weights ----
            wqkv_sb = constp.tile([128, 8, 192], F16)
            nc.sync.dma_start(
                out=wqkv_sb[:], in_=wqkv.rearrange("(c p) m -> p c m", p=128)
            )
            mask_sb = constp.tile([128, T], F16)
            nc.sync.dma_start(out=mask_sb[:], in_=mask[:])

            identf = constp.tile([128, 128], F32)
            make_identity(nc, identf[:])
            ident16 = constp.tile([128, 128], F16)
            nc.vector.tensor_copy(ident16[:], identf[:])

            qstage = constp.tile([128, NPAIR * 16], F16)
            bias_tc = constp.tile([128, B, T], F16)

            # ---- x load (4 tiles of 2 chunks for early matmul start) ----
            xts = []
            xv = xT.rearrange("(c p) r -> p c r", p=128)
            for i in range(4):
                xt = xtp.tile([128, 2, B * TC], F16, tag=f"xt{i}", name=f"xt{i}")
                nc.scalar.dma_start(out=xt[:], in_=xv[:, 2 * i : 2 * i + 2, :])
                xts.append(xt)

            # ---- q,k projections: psQK[128=(q|k), B*TC] ----
            psQK = psA.tile([128, B * TC], F32, tag="big", name="psQK")
            for h2 in range(2):
                sl = slice(512 * h2, 512 * h2 + 512)
                for c in range(8):
                    nc.tensor.matmul(
                        psQK[:, sl],
                        wqkv_sb[:, c, 0:128],
                        xts[c // 2][:, c % 2, sl],
                        start=(c == 0),
                        stop=(c == 7),
                    )
            qT16 = constp.tile([H, B * TC], F16)
            kT16 = constp.tile([H, B * TC], F16)
            nc.scalar.activation(qT16[:], psQK[0:H, :], Copy)
            nc.scalar.activation(kT16[:], psQK[H:128, :], Copy, scale=8.0)

            # ---- v projection: vT[64, B*TC] then transpose to [tl, b, h] ----
            psVT_t = psA.tile([128, B * TC], F32, tag="big", name="psVT")
            psVT = psVT_t[0:H]
            for h2 in range(2):
                sl = slice(512 * h2, 512 * h2 + 512)
                for c in range(8):
                    nc.tensor.matmul(
                        psVT[:, sl],
                        wqkv_sb[:, c, 128:192],
                        xts[c // 2][:, c % 2, sl],
                        start=(c == 0),
                        stop=(c == 7),
                    )
            vT16 = constp.tile([H, B * TC], F16)
            nc.scalar.activation(vT16[:], psVT, Copy)
            v_sb = constp.tile([128, B, H], F16)
            if VT_DMA_T:
                for b in range(B):
                    nc.scalar.dma_start_transpose(
                        out=v_sb[:, b, :], in_=vT16[:, b * TC : (b + 1) * TC]
                    )
            else:
                for b in range(B):
                    psTr_t = psA.tile([128, B * TC], F32, tag="big", name=f"vtr{b}")
                    psTr = psTr_t.bitcast(F16)
                    nc.tensor.transpose(
                        psTr[:, 0:H], vT16[:, b * TC : (b + 1) * TC],
                        ident16[0:H, 0:H],
                    )
                    nc.scalar.activation(v_sb[:, b, :], psTr[:, 0:H], Copy)

            if DEBUG:
                nc.scalar.dma_start(out=dbg["d_qk"][0:H, :], in_=qT16[:])
                nc.scalar.dma_start(out=dbg["d_qk"][H:128, :], in_=kT16[:])
                nc.scalar.dma_start(
                    out=dbg["d_vsb"][:],
                    in_=v_sb[:].rearrange("p b h -> p (b h)"),
                )

            # ---- kick all-gather of (kT, v) in f16 ----
            nc.sync.dma_start(out=cc_in[0:H, :], in_=kT16[:])
            nc.sync.dma_start(
                out=cc_in[H:TC, :].rearrange("p (a bh) -> (p a) bh", a=2),
                in_=v_sb[:].rearrange("p b h -> p (b h)"),
            )
            nc.gpsimd.collective_compute(
                "AllGather",
                mybir.AluOpType.bypass,
                replica_groups=[list(range(num_cores))],
                ins=[cc_in[:]],
                outs=[cc_out[:]],
            )
            # memsets after CC emission so gpsimd reaches the collective
            # barrier as early as possible
            nc.gpsimd.memset(qstage[:], 0.0)
            nc.gpsimd.memset(bias_tc[:], 0.0)
            # post-CC loads on gpsimd: same engine as the collective, so they
            # are program-ordered after it (DRAM is not hazard-tracked).
            # Split per b so attention b can start as soon as its slice lands.
            kT_all = constp.tile([H, NC, B, TC], F16)
            v_nat = constp.tile([TC, NC, B, H], F16)
            cc_k = cc_out.rearrange("(j p) (b t) -> p j b t", j=NC, p=TC, b=B)
            cc_v = cc_out.rearrange(
                "(j w p) (a b h) -> (p a) j w b h", j=NC, w=2, p=H, a=2, b=B
            )
            nc.gpsimd.dma_start(out=kT_all[:], in_=cc_k[0:H])
            nc.gpsimd.dma_start(out=v_nat[:], in_=cc_v[:, :, 1])

            if DEBUG:
                nc.scalar.dma_start(out=dbg["d_ccout"][:], in_=cc_out[:])

            # ---- qstage: col(p,s,b) = q[b, row 2p+s], partitions s*64..+64 ----
            qsrc = qT16.rearrange("h (b p s) -> h p s b", b=B, p=NPAIR, s=2)
            qdst_lo = qstage[0:H, :].rearrange("h (p s b) -> h p s b", p=NPAIR, s=2, b=B)
            qdst_hi = qstage[H:128, :].rearrange("h (p s b) -> h p s b", p=NPAIR, s=2, b=B)
            nc.vector.tensor_copy(qdst_lo[:, :, 0, :], qsrc[:, :, 0, :])
            nc.vector.tensor_copy(qdst_hi[:, :, 1, :], qsrc[:, :, 1, :])

            # ---- bias phase: 16 chunks of 4 pairs ----
            for i in range(16):
                ext = CHUNK_EXT[i]
                rchunk = rpools[i // 4].tile([128, 4, ext], F16, tag=f"rc{i // 4}")
                c0 = int(PAIR_OFF[4 * i])
                nc.sync.dma_start(
                    out=rchunk[:],
                    in_=relp[:, c0 : c0 + 4 * ext].rearrange(
                        "p (f e) -> p f e", f=4
                    ),
                )
                psBias = psA.tile([128, B * TC], F32, tag="big", name=f"psB{i}")
                for p4 in range(4):
                    p = 4 * i + p4
                    for e0 in range(0, ext, 512):
                        e1 = min(ext, e0 + 512)
                        nc.tensor.matmul(
                            psBias[32 * p4 : 32 * p4 + 16, e0:e1],
                            qstage[:, 16 * p : 16 * p + 16],
                            rchunk[:, p4, e0:e1],
                            tile_position=(0, 32 * p4),
                            start=True,
                            stop=True,
                        )
                # quad copy with j-slot scatter on DVE (slot width W per
                # class so attention does wide uniform adds)
                kcl = i // 4
                Wk = 64 if kcl < 2 else 128
                tk = 32 * (kcl + 1)
                bias_sb = bstp.tile([128, T], F16, tag="bsb")
                if i < 2:
                    # first use of each buf: zero (uninit SBUF may hold NaN
                    # bit patterns; later stale data is finite and masked)
                    nc.gpsimd.memset(bias_sb[:], 0.0)
                nc.vector.tensor_copy(
                    bias_sb[:, 0 : 8 * Wk].rearrange("u (j t) -> u j t", j=NC)[
                        :, :, 0:tk
                    ],
                    psBias[:, 0:ext].rearrange("u (j t) -> u j t", j=NC),
                )
                # partition redistribution via compact DRAM stage (only the
                # 64 useful band rows). All on one queue: DRAM write->read
                # ordering relies on queue FIFO.
                r0 = 8 * i
                o = int(ST_OFF[i])
                nc.scalar.dma_start(
                    out=stage[o : o + 128 * 8 * Wk].rearrange(
                        "(r v) -> r v", r=128
                    ),
                    in_=bias_sb[:, 0 : 8 * Wk],
                )
                st_v = stage[o : o + 128 * 8 * Wk].rearrange(
                    "(u s b v) -> u s b v", u=8, s=2, b=B
                )
                for p4 in range(4):
                    nc.scalar.dma_start(
                        out=bias_tc[r0 + 2 * p4 : r0 + 2 * p4 + 2, :, 0 : 8 * Wk],
                        in_=st_v[2 * p4],
                    )

            if DEBUG:
                nc.scalar.dma_start(
                    out=dbg["d_biastc"][:],
                    in_=bias_tc[:].rearrange("p b v -> p (b v)"),
                )

            # ---- attention phase: per batch b ----
            for b in range(B):
                psS = psA.tile([128, T], F32, tag="big", name=f"psS{b}")
                # preload causal mask into PSUM; scores accumulate on top
                nc.scalar.activation(psS[:], mask_sb[:], Copy)
                for h2 in range(2):
                    nc.tensor.matmul(
                        psS[:, 512 * h2 : 512 * h2 + 512],
                        qT16[:, b * TC : (b + 1) * TC],
                        kT_all[:, 4 * h2 : 4 * h2 + 4, b, :],
                        start=False,
                        stop=True,
                        skip_group_check=True,
                    )
                # bias add in PSUM: lower rows slot width 64, upper full 128
                psS_v = psS.rearrange("p (j t) -> p j t", j=NC)
                nc.vector.tensor_tensor(
                    out=psS_v[0:64, :, 0:64],
                    in0=psS_v[0:64, :, 0:64],
                    in1=bias_tc[0:64, b, 0:512].rearrange(
                        "p (j t) -> p j t", j=NC
                    ),
                    op=mybir.AluOpType.add,
                )
                nc.vector.tensor_tensor(
                    out=psS[64:128, :],
                    in0=psS[64:128, :],
                    in1=bias_tc[64:128, b, :],
                    op=mybir.AluOpType.add,
                )
                negmax = smallp.tile([128, 1], F32, tag="nmax")
                nc.vector.reduce_max(
                    negmax[:], psS[:], axis=mybir.AxisListType.X, negate=True
                )
                attn_e = attnp.tile([128, T], F16, tag="aexp")
                denom = smallp.tile([128, 1], F32, tag="den")
                nc.scalar.activation(
                    attn_e[:], psS[:], Exp,
                    bias=negmax[:], scale=1.0, accum_out=denom[:],
                )
                attnT = attnp.tile([128, T], F16, tag="aT")
                if ATTN_DMA_T:
                    nc.sync.dma_start_transpose(
                        out=attnT[:].rearrange("p (j f) -> p j f", j=NC),
                        in_=attn_e[:],
                    )
                else:
                    for g2 in range(2):
                        psT = psTp.tile([128, 512], F16, tag="pt", name=f"pt{b}{g2}")
                        for s4 in range(4):
                            j = 4 * g2 + s4
                            nc.tensor.transpose(
                                psT[:, 128 * s4 : 128 * s4 + 128],
                                attn_e[:, 128 * j : 128 * j + 128],
                                ident16[:],
                            )
                        nc.scalar.activation(
                            attnT[:, 512 * g2 : 512 * g2 + 512], psT[:], Copy
                        )
                if DEBUG and b == 0:
                    nc.scalar.dma_start(out=dbg["d_ae"][:], in_=attn_e[:])
                    nc.scalar.dma_start(out=dbg["d_aT"][:], in_=attnT[:])
                psO = psOp.tile([128, H], F32, tag="o", name=f"psO{b}")
                for j in range(8):
                    nc.tensor.matmul(
                        psO[:],
                        attnT[:, 128 * j : 128 * j + 128],
                        v_nat[:, j, b, :],
                        start=(j == 0),
                        stop=(j == 7),
                    )
                rden = smallp.tile([128, 1], F32, tag="rden")
                nc.vector.reciprocal(rden[:], denom[:])
                out_sb = smallp.tile([128, H], F32, tag="osb")
                nc.scalar.activation(out_sb[:], psO[:], Copy, scale=rden[:])
                nc.sync.dma_start(
                    out=out_e[b * TC : (b + 1) * TC, :], in_=out_sb[:]
                )
    nc.compile()
    return nc


_CACHE: dict = {}


def _get_nc():
    if "nc" not in _CACHE:
        _CACHE["nc"] = build(NC)
    return _CACHE["nc"]


def _prep_inputs(x, Wq, Wk, Wv, relpos):
    x16 = np.asarray(x, dtype=np.float16)
    rel16 = np.asarray(relpos, dtype=np.float16)
    wqkv = np.ascontiguousarray(
        np.concatenate([Wq, Wk, Wv], axis=1), dtype=np.float16
    )
    # column order of the gathered k/v axis: col j*128+tl -> global_rows(j)[tl]
    all_rows = [global_rows(j) for j in range(NC)]
    vperm = np.concatenate(all_rows)
    # per class k, the packed bias columns: concat_j global_rows(j)[:32*(k+1)]
    vpack = [
        np.concatenate([all_rows[j][: 32 * (k + 1)] for j in range(NC)])
        for k in range(4)
    ]
    in_maps = []
    for c in range(NC):
        rows = all_rows[c]
        xs = x16[:, rows, :]  # [B, 128, D]
        xT = np.ascontiguousarray(xs.transpose(2, 0, 1).reshape(D, B * TC))
        # relp stream: per pair p, [128, ext] = [R_tA^T ; R_tB^T] with columns
        # in the packed per-class order (prefix of each core's shard)
        rp = np.zeros((TC, RELP_COLS), dtype=np.float16)
        for p in range(NPAIR):
            tA = int(rows[2 * p])
            tB = int(rows[2 * p + 1])
            vr = vpack[p // 16]
            o = int(PAIR_OFF[p])
            e = PAIR_EXT[p]
            rp[0:H, o : o + e] = rel16[tA, vr, :].T
            rp[H:128, o : o + e] = rel16[tB, vr, :].T
        # mask in gathered column order
        msk = np.where(vperm[None, :] <= rows[:, None], 0.0, NEG).astype(
            np.float16
        )
        in_maps.append({"xT": xT, "wqkv": wqkv, "relp": rp, "mask": msk})
    return in_maps


def run_sharded(in_maps, trace=False, **kw):
    nc = _get_nc()
    return run_bass_kernel_spmd(
        nc, in_maps, core_ids=list(range(NC)), trace=trace, **kw
    )


def kernel(x, Wq, Wk, Wv, relpos):
    in_maps = _prep_inputs(x, Wq, Wk, Wv, relpos)
    res = run_sharded(in_maps, trace=False)
    out = np.empty((B, T, H), dtype=np.float32)
    for c in range(NC):
        rows = global_rows(c)
        out[:, rows, :] = res.results[c]["out"].reshape(B, TC, H)
    return out
